# revision 15
# baseline (speedup 1.0000x reference)
import sys
sys.path.insert(0, "/opt/trn_rl_repo")
import numpy as np
import ml_dtypes

import concourse.bass as bass
import concourse.mybir as mybir
from concourse import bacc
from concourse.tile import TileContext
from concourse.masks import make_identity

f32 = mybir.dt.float32
bf16 = mybir.dt.bfloat16

# Model constants (hardcoded per problem spec)
B, S, V, E, H = 64, 256, 50000, 300, 256
LBL = 20
T = LBL + 2           # 22
START, STOP = T - 2, T - 1
NC = 8
BL = B // NC          # 8 sequences per core
TOK = BL * S          # 2048 tokens per core (b-major flat order)
EP = 384              # E padded to 3 x 128
G4 = 4 * H            # 1024 gate dims per direction
RENORM = 8
BIG = 1.0e6

_cache = {}


def _build():
    nc = bacc.Bacc(None, target_bir_lowering=False, debug=False)

    # ---- inputs ----
    emb = nc.dram_tensor("emb", [V, E], f32, kind="ExternalInput")
    tok = nc.dram_tensor("tok", [TOK, 1], mybir.dt.int32, kind="ExternalInput")
    wih = nc.dram_tensor("wih", [2, 3, 128, G4], bf16, kind="ExternalInput")
    whh = nc.dram_tensor("whh", [2, 2, 128, G4], bf16, kind="ExternalInput")
    bias2 = nc.dram_tensor("bias2", [128, 2, 8], f32, kind="ExternalInput")
    wtag = nc.dram_tensor("wtag", [4, 128, 64], bf16, kind="ExternalInput")
    btag = nc.dram_tensor("btag", [64, 1], f32, kind="ExternalInput")
    etr = nc.dram_tensor("etr", [22, 22], f32, kind="ExternalInput")
    estart = nc.dram_tensor("estart", [22, 1], f32, kind="ExternalInput")
    estop = nc.dram_tensor("estop", [22, 1], f32, kind="ExternalInput")
    ones22 = nc.dram_tensor("ones22", [22, 1], f32, kind="ExternalInput")
    onesr = nc.dram_tensor("onesr", [1, 22], f32, kind="ExternalInput")
    trrep = nc.dram_tensor("trrep", [8, 484], f32, kind="ExternalInput")
    trrep128 = nc.dram_tensor("trrep128", [128, 484], f32, kind="ExternalInput")
    tstart8 = nc.dram_tensor("tstart8", [8, 22], f32, kind="ExternalInput")
    tstop8 = nc.dram_tensor("tstop8", [128, 22], f32, kind="ExternalInput")
    iota6 = nc.dram_tensor("iota6", [128, 484], f32, kind="ExternalInput")
    ohT = nc.dram_tensor("ohT", [22, TOK], f32, kind="ExternalInput")
    cmT = nc.dram_tensor("cmT", [22, BL * 22], f32, kind="ExternalInput")
    transd = nc.dram_tensor("transd", [22, BL * 22], f32, kind="ExternalInput")
    ltok = nc.dram_tensor("ltok", [1, TOK], f32, kind="ExternalInput")

    # ---- outputs ----
    o_logz = nc.dram_tensor("o_logz", [1, BL], f32, kind="ExternalOutput")
    o_gemit = nc.dram_tensor("o_gemit", [22, 1], f32, kind="ExternalOutput")
    o_gcm = nc.dram_tensor("o_gcm", [22, 1], f32, kind="ExternalOutput")
    o_ce = nc.dram_tensor("o_ce", [1, 1], f32, kind="ExternalOutput")
    o_bp = nc.dram_tensor("o_bp", [128, 16, 22], f32, kind="ExternalOutput")
    o_last = nc.dram_tensor("o_last", [8, 8], mybir.dt.uint32, kind="ExternalOutput")

    # scratch DRAM for relayouts
    fdram = nc.dram_tensor("fdram", [BL, S, 22], f32)
    ddram = nc.dram_tensor("ddram", [1, TOK], f32)
    vdram = nc.dram_tensor("vdram", [BL, S, 22], f32)

    with TileContext(nc) as tc:
        with tc.tile_pool(name="wpool", bufs=1) as wp, \
             tc.tile_pool(name="state", bufs=1) as st:

            # --- load weights/constants ---
            wih_sb = wp.tile([128, 2, 3, G4], bf16)
            nc.sync.dma_start(out=wih_sb[:, :, :, :],
                              in_=wih[:, :, :, :].rearrange("d e p g -> p d e g"))
            whh_sb = wp.tile([128, 2, 2, G4], bf16)
            nc.sync.dma_start(out=whh_sb[:, :, :, :],
                              in_=whh[:, :, :, :].rearrange("d k p g -> p d k g"))
            bias_sb = wp.tile([128, 2, 8], f32)
            nc.sync.dma_start(out=bias_sb[:, :, :], in_=bias2[:, :, :])
            wtag_sb = wp.tile([128, 4, 64], bf16)
            nc.sync.dma_start(out=wtag_sb[:, :, :],
                              in_=wtag[:, :, :].rearrange("k p m -> p k m"))
            btag_sb = wp.tile([64, 1], f32)
            nc.sync.dma_start(out=btag_sb[:, :], in_=btag[:, :])
            etr_sb = wp.tile([22, 22], f32)
            nc.sync.dma_start(out=etr_sb[:, :], in_=etr[:, :])
            estart_sb = wp.tile([22, 1], f32)
            nc.sync.dma_start(out=estart_sb[:, :], in_=estart[:, :])
            estop_sb = wp.tile([22, 1], f32)
            nc.sync.dma_start(out=estop_sb[:, :], in_=estop[:, :])
            ones22_sb = wp.tile([22, 1], f32)
            nc.sync.dma_start(out=ones22_sb[:, :], in_=ones22[:, :])
            onesr_sb = wp.tile([1, 22], f32)
            nc.sync.dma_start(out=onesr_sb[:, :], in_=onesr[:, :])
            trrep_sb = wp.tile([8, 484], f32)
            nc.sync.dma_start(out=trrep_sb[:, :], in_=trrep[:, :])
            trrep128_sb = wp.tile([128, 484], f32)
            nc.sync.dma_start(out=trrep128_sb[:, :], in_=trrep128[:, :])
            tstart8_sb = wp.tile([8, 22], f32)
            nc.sync.dma_start(out=tstart8_sb[:, :], in_=tstart8[:, :])
            tstop8_sb = wp.tile([128, 22], f32)
            nc.sync.dma_start(out=tstop8_sb[:, :], in_=tstop8[:, :])
            iota6_sb = wp.tile([128, 484], f32)
            nc.sync.dma_start(out=iota6_sb[:, :], in_=iota6[:, :])
            ohT_sb = wp.tile([22, TOK], f32)
            nc.sync.dma_start(out=ohT_sb[:, :], in_=ohT[:, :])
            cmT_sb = wp.tile([22, BL * 22], f32)
            nc.sync.dma_start(out=cmT_sb[:, :], in_=cmT[:, :])
            transd_sb2 = wp.tile([22, BL * 22], f32)
            nc.sync.dma_start(out=transd_sb2[:, :], in_=transd[:, :])
            ltok_sb = wp.tile([1, TOK], f32, tag="ltok_sb")
            nc.sync.dma_start(out=ltok_sb[:, :], in_=ltok[:, :])
            tok_sb = wp.tile([128, 16], mybir.dt.int32)
            nc.sync.dma_start(out=tok_sb[:, :],
                              in_=tok[:, :].rearrange("(n p) o -> p (n o)", p=128))
            ident = wp.tile([128, 128], f32)
            make_identity(nc, ident)

            # persistent state
            hall = st.tile([128, 4, BL, S], bf16)      # [h-sub, dir*ktile, b, t]
            xproj = st.tile([128, 2, 8, TOK], bf16)    # [g-sub, dir, chunk, (b t)]
            featsT = st.tile([64, BL, S], f32, tag="featsT")

            # --- phase 1: gather + transpose x ---
            with tc.tile_pool(name="xp", bufs=3) as xp, \
                 tc.tile_pool(name="pst", bufs=4, space="PSUM") as pst:
                xT = xp.tile([128, 3, TOK], bf16, tag="xT", bufs=1)
                nc.vector.memset(xT[:, 2, :], 0.0)
                for tt in range(16):
                    xt = xp.tile([128, E], f32, tag="gath")
                    nc.gpsimd.indirect_dma_start(
                        out=xt[:, :], out_offset=None, in_=emb[:, :],
                        in_offset=bass.IndirectOffsetOnAxis(ap=tok_sb[:, tt:tt + 1],
                                                            axis=0))
                    for ec in range(3):
                        w = min(128, E - ec * 128)
                        pt = pst.tile([128, 128], f32, tag="ptr")
                        nc.tensor.transpose(pt[0:w, :], xt[:, ec * 128:ec * 128 + w],
                                            ident[:, :])
                        nc.vector.tensor_copy(xT[0:w, ec, tt * 128:(tt + 1) * 128],
                                              pt[0:w, :])


                # --- phase 2: xproj = Wih @ x (+bias), both dirs ---
                for d in range(2):
                    for m in range(8):
                        for ntile in range(4):
                            pxm = pst.tile([128, 512], f32, tag="px")
                            for ec in range(3):
                                nc.tensor.matmul(
                                    pxm[:, :],
                                    wih_sb[:, d, ec, m * 128:(m + 1) * 128],
                                    xT[:, ec, ntile * 512:(ntile + 1) * 512],
                                    start=(ec == 0), stop=(ec == 2))
                            nc.vector.tensor_scalar_add(
                                xproj[:, d, m, ntile * 512:(ntile + 1) * 512],
                                pxm[:, :], bias_sb[:, d, m:m + 1])

            # --- phase 3: LSTM scans (fwd + bwd interleaved) ---
            with tc.tile_pool(name="lst", bufs=4) as lp, \
                 tc.tile_pool(name="psg", bufs=4, space="PSUM") as psg:
                c_sb = st.tile([128, 2, 2, BL], f32)   # [h-sub, dir, ktile, b]
                for s in range(S):
                    for d in range(2):
                        treal = s if d == 0 else S - 1 - s
                        tprev = treal - 1 if d == 0 else treal + 1
                        g_sb = lp.tile([128, 8, BL], f32, tag=f"g{d}")
                        if s == 0:
                            nc.vector.tensor_copy(
                                g_sb[:, :, :],
                                xproj[:, d, :, :].rearrange(
                                    "p m (b t) -> p m b t", b=BL)[:, :, :, treal])
                        else:
                            pg = psg.tile([128, 8, BL], f32, tag=f"pg{d}")
                            for m in range(8):
                                for kt in range(2):
                                    nc.tensor.matmul(
                                        pg[:, m, :],
                                        whh_sb[:, d, kt, m * 128:(m + 1) * 128],
                                        hall[:, 2 * d + kt, :, tprev],
                                        start=(kt == 0), stop=(kt == 1))
                            nc.vector.scalar_tensor_tensor(
                                out=g_sb[:, :, :], in0=pg[:, :, :], scalar=0.0,
                                in1=xproj[:, d, :, :].rearrange(
                                    "p m (b t) -> p m b t", b=BL)[:, :, :, treal],
                                op0=mybir.AluOpType.add, op1=mybir.AluOpType.add)
                        act = lp.tile([128, 8, BL], f32, tag=f"a{d}")
                        nc.scalar.activation(act[:, 0:6, :], g_sb[:, 0:6, :],
                                             mybir.ActivationFunctionType.Sigmoid)
                        nc.scalar.activation(act[:, 6:8, :], g_sb[:, 6:8, :],
                                             mybir.ActivationFunctionType.Tanh)
                        m1 = lp.tile([128, 2, BL], f32, tag=f"m{d}")
                        nc.vector.tensor_mul(m1[:, :, :], act[:, 0:2, :], act[:, 6:8, :])
                        if s == 0:
                            nc.vector.tensor_copy(c_sb[:, d, :, :], m1[:, :, :])
                        else:
                            c2 = lp.tile([128, 2, BL], f32, tag=f"c2{d}")
                            nc.gpsimd.tensor_mul(c2[:, :, :], act[:, 2:4, :],
                                                 c_sb[:, d, :, :])
                            nc.vector.tensor_add(c_sb[:, d, :, :], m1[:, :, :],
                                                 c2[:, :, :])
                        tch = lp.tile([128, 2, BL], f32, tag=f"t{d}")
                        nc.scalar.activation(tch[:, :, :], c_sb[:, d, :, :],
                                             mybir.ActivationFunctionType.Tanh)
                        nc.vector.tensor_mul(hall[:, 2 * d:2 * d + 2, :, treal],
                                             act[:, 4:6, :], tch[:, :, :])

            # --- phase 4: tag features ---
            with tc.tile_pool(name="psf", bufs=4, space="PSUM") as psf:
                for ntile in range(4):
                    pf = psf.tile([64, 512], f32, tag="pf")
                    for kt in range(4):
                        nc.tensor.matmul(
                            pf[:, :], wtag_sb[:, kt, :],
                            hall[:, kt, :, :].rearrange("p b t -> p (b t)")[
                                :, ntile * 512:(ntile + 1) * 512],
                            start=(kt == 0), stop=(kt == 3))
                    nc.vector.tensor_scalar_add(
                        featsT[:, :, :].rearrange("p b t -> p (b t)")[
                            :, ntile * 512:(ntile + 1) * 512],
                        pf[:, :], btag_sb[:, 0:1])

            # --- phase 5: F=exp(feats), CE, gold ---
            F_sb = st.tile([22, BL, S], f32)
            nc.scalar.activation(F_sb[:, :, :], featsT[0:22, :, :],
                                 mybir.ActivationFunctionType.Exp)
            ph5_cm = tc.tile_pool(name="ph5", bufs=1)
            ph5 = ph5_cm.__enter__()
            dtmp = ph5.tile([1, TOK], f32, tag="dtmp")
            nc.sync.dma_start(
                out=ddram[:, :],
                in_=featsT[:, :, :].rearrange("p b t -> p (b t)")[32:33, :])
            nc.sync.dma_start(out=dtmp[:, :], in_=ddram[:, :])
            spb = ph5.tile([1, TOK], f32)
            nc.scalar.activation(spb[:, :], dtmp[:, :],
                                 mybir.ActivationFunctionType.Exp, scale=-1.0)
            nc.scalar.activation(spb[:, :], spb[:, :],
                                 mybir.ActivationFunctionType.Ln, bias=1.0)
            ttmp = ph5.tile([1, TOK], f32, tag="ttmp")
            cacc = ph5.tile([1, 1], f32, tag="cacc")
            nc.vector.tensor_mul(ttmp[:, :], spb[:, :], ltok_sb[:, :])
            nc.vector.tensor_reduce(cacc[:, :], ttmp[:, :],
                                    axis=mybir.AxisListType.X,
                                    op=mybir.AluOpType.add)
            nc.sync.dma_start(out=o_ce[:, :], in_=cacc[:, :])

            gtmp = ph5.tile([22, TOK], f32, tag="gtmp")
            gacc = ph5.tile([22, 1], f32, tag="gacc")
            nc.vector.tensor_mul(gtmp[:, :],
                                 featsT[0:22, :, :].rearrange("p b t -> p (b t)"),
                                 ohT_sb[:, :])
            nc.vector.tensor_reduce(gacc[:, :], gtmp[:, :],
                                    axis=mybir.AxisListType.X,
                                    op=mybir.AluOpType.add)
            nc.sync.dma_start(out=o_gemit[:, :], in_=gacc[:, :])

            gtmp2 = ph5.tile([22, BL * 22], f32, tag="gtmp2")
            gacc2 = ph5.tile([22, 1], f32, tag="gacc2")
            nc.vector.tensor_mul(gtmp2[:, :], transd_sb2[:, :], cmT_sb[:, :])
            nc.vector.tensor_reduce(gacc2[:, :], gtmp2[:, :],
                                    axis=mybir.AxisListType.X,
                                    op=mybir.AluOpType.add)
            nc.sync.dma_start(out=o_gcm[:, :], in_=gacc2[:, :])
            ph5_cm.__exit__(None, None, None)

            # --- phase 6: feats -> fb [b, t, j] via DRAM bounce ---
            fb32 = st.tile([128, S // 4, 22], f32)
            for b in range(BL):
                nc.sync.dma_start(
                    out=fdram[b, :, :].rearrange("t j -> j t"),
                    in_=featsT[0:22, b, :])
            for b in range(BL):
                for c in range(4):
                    nc.sync.dma_start(
                        out=fb32[32 * c + b:32 * c + b + 1, :, :],
                        in_=fdram[b, :, :].rearrange("(q c) j -> c q j", c=4)[
                            c].unsqueeze(0))

            # --- phase 7: alpha scan (linear-space CRF forward) ---
            with tc.tile_pool(name="apool", bufs=4) as ap_, \
                 tc.tile_pool(name="psa", bufs=2, space="PSUM") as psa:
                a_sb = st.tile([22, BL], f32)
                nc.vector.tensor_scalar_mul(a_sb[:, :], F_sb[:, :, 0],
                                            estart_sb[:, 0:1])
                lacc = st.tile([1, BL], f32)
                nc.vector.memset(lacc[:, :], 0.0)
                for t in range(1, S):
                    pa = psa.tile([22, BL], f32, tag="pa")
                    nc.tensor.matmul(pa[:, :], etr_sb[:, :], a_sb[:, :],
                                     start=True, stop=True)
                    nc.vector.tensor_mul(a_sb[:, :], pa[:, :], F_sb[:, :, t])
                    if t % RENORM == RENORM - 1 or t == S - 1:
                        ps_ = psa.tile([1, BL], f32, tag="ps")
                        nc.tensor.matmul(ps_[:, :], ones22_sb[:, :], a_sb[:, :],
                                         start=True, stop=True)
                        s_sb = ap_.tile([1, BL], f32, tag="ssb")
                        nc.vector.tensor_copy(s_sb[:, :], ps_[:, :])
                        r_sb = ap_.tile([1, BL], f32, tag="rsb")
                        nc.vector.reciprocal(r_sb[:, :], s_sb[:, :])
                        pb = psa.tile([22, BL], f32, tag="pb")
                        nc.tensor.matmul(pb[:, :], onesr_sb[:, :], r_sb[:, :],
                                         start=True, stop=True)
                        nc.vector.tensor_mul(a_sb[:, :], a_sb[:, :], pb[:, :])
                        lg = ap_.tile([1, BL], f32, tag="lg")
                        nc.scalar.activation(lg[:, :], s_sb[:, :],
                                             mybir.ActivationFunctionType.Ln)
                        nc.vector.tensor_add(lacc[:, :], lacc[:, :], lg[:, :])
                pfin = psa.tile([1, BL], f32, tag="pfin")
                nc.tensor.matmul(pfin[:, :], estop_sb[:, :], a_sb[:, :],
                                 start=True, stop=True)
                sfin = ap_.tile([1, BL], f32, tag="sfin")
                nc.vector.tensor_copy(sfin[:, :], pfin[:, :])
                lgf = ap_.tile([1, BL], f32, tag="lgf")
                nc.scalar.activation(lgf[:, :], sfin[:, :],
                                     mybir.ActivationFunctionType.Ln)
                logz = ap_.tile([1, BL], f32, tag="logz")
                nc.vector.tensor_add(logz[:, :], lgf[:, :], lacc[:, :])
                nc.sync.dma_start(out=o_logz[:, :], in_=logz[:, :])

            # --- phase 8: viterbi scan ---
            with tc.tile_pool(name="vpool", bufs=4) as vp:
                vit32 = st.tile([128, S // 4, 22], f32)

                def vsl(t):
                    c, q = t % 4, t // 4
                    return vit32[32 * c:32 * c + BL, q, :]

                def fsl(t):
                    c, q = t % 4, t // 4
                    return fb32[32 * c:32 * c + BL, q, :]

                nc.vector.tensor_add(vsl(0), fsl(0), tstart8_sb[:, :])
                sc = st.tile([128, 22, 22], f32, tag="scq")
                rmx = st.tile([128, 22], f32, tag="rmxq")
                for t in range(1, S):
                    cp = 32 * ((t - 1) % 4)
                    cc = 32 * (t % 4)
                    nc.vector.tensor_add(
                        sc[cp:cp + BL, :, :],
                        vsl(t - 1).unsqueeze(1).to_broadcast([BL, 22, 22]),
                        trrep128_sb[cp:cp + BL, :].rearrange(
                            "b (j i) -> b j i", j=22))
                    nc.vector.tensor_reduce(rmx[cc:cc + BL, :],
                                            sc[cp:cp + BL, :, :],
                                            axis=mybir.AxisListType.X,
                                            op=mybir.AluOpType.max)
                    nc.gpsimd.tensor_add(vsl(t), rmx[cc:cc + BL, :], fsl(t))
                fin = vp.tile([BL, 22], f32, tag="fin")
                c255 = 32 * ((S - 1) % 4)
                nc.vector.tensor_add(fin[:, :], vsl(S - 1),
                                     tstop8_sb[c255:c255 + BL, :])
                mx8 = vp.tile([BL, 8], f32, tag="mx8")
                mi8 = vp.tile([BL, 8], mybir.dt.uint32, tag="mi8")
                nc.vector.max(mx8[:, :], fin[:, :])
                nc.vector.max_index(mi8[:, :], mx8[:, :], fin[:, :])
                nc.sync.dma_start(out=o_last[:, :], in_=mi8[:, :])

                # bounce vit32 -> vdram [b, t, j]
                for c in range(4):
                    nc.sync.dma_start(
                        out=vdram[:, :, :].rearrange("b (q cc) j -> cc b q j", cc=4)[c],
                        in_=vit32[32 * c:32 * c + BL, :, :])

            # --- phase 9: batched backpointers ---
            with tc.tile_pool(name="bpool", bufs=1) as bp_:
                vit_wide = bp_.tile([128, 16, 22], f32)
                for tsub in range(16):
                    nc.sync.dma_start(
                        out=vit_wide[8 * tsub:8 * tsub + 8, :, :],
                        in_=vdram[:, :, :].rearrange("b (tt u) j -> u b tt j", u=16)[
                            tsub, :, :, :].rearrange("b tt j -> b tt j"))
                for hh in range(2):
                    hs = slice(hh * 8, hh * 8 + 8)
                    sca = bp_.tile([128, 8, 22, 22], f32, tag="sca")
                    nc.vector.tensor_add(
                        sca[:, :, :, :],
                        vit_wide[:, hs, :].unsqueeze(2).to_broadcast(
                            [128, 8, 22, 22]),
                        trrep128_sb[:, :].rearrange("p (j i) -> p j i", j=22)
                            .unsqueeze(1).to_broadcast([128, 8, 22, 22]))
                    mall = bp_.tile([128, 8, 22], f32, tag="mall")
                    nc.vector.tensor_reduce(mall[:, :, :], sca[:, :, :, :],
                                            axis=mybir.AxisListType.X,
                                            op=mybir.AluOpType.max)
                    eqa = bp_.tile([128, 8, 22, 22], f32, tag="eqa")
                    nc.vector.tensor_tensor(
                        eqa[:, :, :, :], sca[:, :, :, :],
                        mall[:, :, :].unsqueeze(3).to_broadcast([128, 8, 22, 22]),
                        op=mybir.AluOpType.is_equal)
                    bpf = bp_.tile([128, 8, 484], f32, tag="sca")
                    nc.vector.scalar_tensor_tensor(
                        out=bpf[:, :, :],
                        in0=eqa[:, :, :, :].rearrange("p a j i -> p a (j i)"),
                        scalar=-BIG,
                        in1=iota6_sb[:, :].unsqueeze(1).to_broadcast([128, 8, 484]),
                        op0=mybir.AluOpType.mult, op1=mybir.AluOpType.add)
                    bpv = bp_.tile([128, 8, 22], f32, tag="bpv")
                    nc.vector.tensor_reduce(bpv[:, :, :],
                                            bpf[:, :, :].rearrange(
                                                "p a (j i) -> p a j i", j=22),
                                            axis=mybir.AxisListType.X,
                                            op=mybir.AluOpType.min)
                    nc.sync.dma_start(out=o_bp[:, hs, :], in_=bpv[:, :, :])

    nc.finalize()
    return nc


def _prep_inputs(inputs):
    """Host-side input prep: slicing, weight reshuffles, one-hots."""
    f = np.float32
    emb = np.asarray(inputs["emb_table"], f)
    word = np.asarray(inputs["word_input"])
    labels = np.asarray(inputs["labels"])
    ltok = np.asarray(inputs["labels_token"])
    trans = np.asarray(inputs["trans"], f)

    perm = np.concatenate([np.arange(0, H), np.arange(H, 2 * H),
                           np.arange(3 * H, 4 * H), np.arange(2 * H, 3 * H)])

    def prep_dir(Wih, Whh, bih, bhh):
        Wih_r = np.asarray(Wih, f)[perm]                      # [1024, 300]
        WihT = np.zeros((EP, G4), f)
        WihT[:E] = Wih_r.T
        Whh_r = np.asarray(Whh, f)[perm]                      # [1024, 256]
        WhhT = Whh_r.T                                        # [256, 1024]
        bias = (np.asarray(bih, f) + np.asarray(bhh, f))[perm]
        return (WihT.reshape(3, 128, G4).astype(ml_dtypes.bfloat16),
                WhhT.reshape(2, 128, G4).astype(ml_dtypes.bfloat16),
                bias.reshape(8, 128).T.copy())                # [128, 8]

    wf = prep_dir(inputs["Wih_f"], inputs["Whh_f"], inputs["bih_f"], inputs["bhh_f"])
    wb = prep_dir(inputs["Wih_b"], inputs["Whh_b"], inputs["bih_b"], inputs["bhh_b"])
    wih = np.stack([wf[0], wb[0]])                            # [2, 3, 128, G4]
    whh = np.stack([wf[1], wb[1]])
    bias2 = np.stack([wf[2], wb[2]], axis=1)                  # [128, 2, 8]

    W_tag = np.asarray(inputs["W_tag"], f)                    # [22, 512]
    W_tok = np.asarray(inputs["W_tok"], f)                    # [2, 512]
    b_tag = np.asarray(inputs["b_tag"], f)
    b_tok = np.asarray(inputs["b_tok"], f)
    Wc = np.zeros((64, 2 * H), f)
    Wc[0:22] = W_tag
    Wc[32] = W_tok[1] - W_tok[0]
    wtag = Wc.T.reshape(4, 128, 64).astype(ml_dtypes.bfloat16)
    btag = np.zeros((64, 1), f)
    btag[0:22, 0] = b_tag
    btag[32, 0] = b_tok[1] - b_tok[0]

    etr = np.exp(trans).astype(f)
    estart = np.exp(trans[START]).reshape(22, 1).astype(f)
    estop = np.exp(trans[:, STOP]).reshape(22, 1).astype(f)
    ones22 = np.ones((22, 1), f)
    onesr = np.ones((1, 22), f)
    trrep = np.tile(trans.T.reshape(1, -1), (8, 1)).astype(f)
    trrep128 = np.tile(trans.T.reshape(1, -1), (128, 1)).astype(f)
    tstart8 = np.tile(trans[START].reshape(1, -1), (8, 1)).astype(f)
    tstop8 = np.tile(trans[:, STOP].reshape(1, -1), (128, 1)).astype(f)
    iota6 = np.tile((np.arange(22, dtype=f) + BIG).reshape(1, 1, 22),
                    (128, 22, 1)).reshape(128, 484)

    base = dict(emb=emb, wih=wih, whh=whh, bias2=bias2, wtag=wtag, btag=btag,
                etr=etr, estart=estart, estop=estop, ones22=ones22, onesr=onesr,
                trrep=trrep, trrep128=trrep128, tstart8=tstart8, tstop8=tstop8,
                iota6=iota6,
                transd=np.tile(trans, (1, BL)).astype(f))

    in_maps = []
    for c in range(NC):
        bs = slice(c * BL, (c + 1) * BL)
        w_c = word[bs].astype(np.int32)                       # [BL, S]
        lab_c = labels[bs].astype(np.int64)
        lt_c = ltok[bs].astype(np.int64)
        tok_c = w_c.reshape(TOK, 1)
        ohT_c = (lab_c.reshape(1, BL, S) ==
                 np.arange(22).reshape(22, 1, 1)).astype(f).reshape(22, TOK)
        prev = np.concatenate([np.full((BL, 1), START, np.int64), lab_c[:, :-1]],
                              axis=1)
        cm = np.zeros((22, BL, 22), f)
        for b in range(BL):
            np.add.at(cm, (prev[b], b, lab_c[b]), 1.0)
            cm[lab_c[b, -1], b, STOP] += 1.0
        cmT_c = cm.reshape(22, BL * 22)
        ltok_c = lt_c.astype(f).reshape(1, TOK)
        m = dict(base)
        m.update(tok=tok_c, ohT=ohT_c, cmT=cmT_c, ltok=ltok_c)
        in_maps.append(m)
    return in_maps


def _postprocess(results, inputs):
    ltok = np.asarray(inputs["labels_token"])
    logz_sum = 0.0
    gold_sum = 0.0
    ce_sum = 0.0
    tag_seq = np.zeros((B, S), np.int32)
    for c, r in enumerate(results):
        logz_sum += float(np.asarray(r["o_logz"], np.float64).sum())
        r = {k: np.asarray(v) for k, v in r.items()}
        r["o_last"] = r["o_last"].reshape(8, 8)
        gold_sum += float(np.asarray(r["o_gemit"], np.float64).sum())
        gold_sum += float(np.asarray(r["o_gcm"], np.float64).sum())
        ce_sum += float(np.asarray(r["o_ce"], np.float64).sum())
        # backtrace
        bpw = np.asarray(r["o_bp"]).reshape(128, 16, 22)
        bp = np.zeros((S, BL, 22), np.int64)
        for tsub in range(16):
            for b in range(BL):
                bp[tsub + 16 * np.arange(16), b, :] = \
                    bpw[8 * tsub + b, :, :].astype(np.int64)
        # bp[t] holds argmax over i of vit[t] + trans -> backpointer INTO t
        # for state at t+1
        last = np.asarray(r["o_last"])[:, 0].astype(np.int64)  # [BL]
        tags = np.zeros((BL, S), np.int64)
        tags[:, S - 1] = last
        for t in range(S - 2, -1, -1):
            tags[:, t] = bp[t, np.arange(BL), tags[:, t + 1]]
        tag_seq[c * BL:(c + 1) * BL] = tags.astype(np.int32)
    w_sum = float(((np.asarray(inputs["input_mask"]) == 1) & (ltok != 0)).sum())
    nll = logz_sum - gold_sum
    token_loss = ce_sum / max(w_sum, 1.0)
    ans_loss = np.float32(nll / B + token_loss)
    return ans_loss, tag_seq


def kernel(**inputs):
    from concourse.bass_utils import run_bass_kernel_spmd
    if "nc" not in _cache:
        _cache["nc"] = _build()
    nc = _cache["nc"]
    in_maps = _prep_inputs(inputs)
    res = run_bass_kernel_spmd(nc, in_maps, core_ids=list(range(NC)))
    return _postprocess(res.results, inputs)
